# revision 18
# baseline (speedup 1.0000x reference)
"""Single-head attention kernel for Trainium2 (Bass/Tile), 8 NeuronCores.

Problem: B=4, S=4096, D=1024, H=128 fp32.
    q,k,v = x @ W{q,k,v};  out = softmax(q k^T / sqrt(H)) @ v

Sharding: 8 cores = (batch b, query-half qh).  Each core computes attention
for 2048 queries over all 4096 keys of one batch element.  The host permutes
each core's x rows so its query rows come first (softmax over keys is
permutation-invariant, so K/V row order does not matter), transposes it to
xT [D, S] (so the kernel never needs PE transposes of x), and casts to fp16.

fp16 (e5m10) is used for all matmul operands: 2-byte operands stream through
the PE at 1 column/cycle @ 2.4 GHz (4-byte fp32/fp32r streams at half rate),
and its 10-bit mantissa keeps the end-to-end error ~5e-4 (all tensors here
are O(1) so the e5 range is ample).  All accumulation is fp32 in PSUM.

Per-core dataflow:
  1. xT d-chunks DMA'd straight into SBUF (fp16, host-prepared).
  2. Projections loop proj-major: for each (proj, d-chunk) one LDWEIGHTS of
     the 128x128 W block, then 8 matmuls (one per 512-row block) accumulating
     into 8 PSUM banks -> 24 weight loads total.  PSUM -> SBUF casts to fp16.
     vT additionally PE-transposed (32 blocks) to v-natural [k,128h].
  3. Scores computed TRANSPOSED in chunks of 1024 queries:
     sT[k, q] = (kT block)^T @ qT -> PSUM [128,1024].
     ScalarE exp reads sT from PSUM, writes attnT (fp16) straight to SBUF --
     softmax PSUM evacuation fused into the exp, no copies.  No max
     subtraction: scores are ~N(0,1) by construction, exp is safe in fp32.
  4. outT[h,q] += v[kb]^T @ attnT, and row-sums l[q] += ones^T @ attnT,
     both accumulated over kb in PSUM (exact fp32).
  5. outT (unnormalized) and l DMA'd out; host does out = (outT/l)^T.
"""

import math

import numpy as np

import concourse.bacc as bacc
import concourse.mybir as mybir
import concourse.tile as tile
from concourse.bass_utils import run_bass_kernel_spmd

B, S, D, H = 4, 4096, 1024, 128
NCORES = 8
SQ = S // 2  # queries per core (2048)
RB = 512  # rows per projection block
NRB = S // RB  # 8
NQRB = SQ // RB  # 4 blocks that need qT
QC = 1024  # queries per attention chunk
NQC = SQ // QC  # 2 chunks
NKB = S // 128  # 32 key blocks
NDC = D // 128  # 8 contraction chunks

F32 = mybir.dt.float32
F16 = mybir.dt.float16

_CACHE = {}


def build_nc():
    nc = bacc.Bacc("TRN2", target_bir_lowering=False, debug=False)

    xt_d = nc.dram_tensor("xt", [D, S], F16, kind="ExternalInput")
    wq_d = nc.dram_tensor("wq", [D, H], F16, kind="ExternalInput")
    wk_d = nc.dram_tensor("wk", [D, H], F16, kind="ExternalInput")
    wv_d = nc.dram_tensor("wv", [D, H], F16, kind="ExternalInput")
    ident_d = nc.dram_tensor("ident", [128, 128], F16, kind="ExternalInput")
    ones_d = nc.dram_tensor("ones", [128, 1], F16, kind="ExternalInput")
    # unnormalized out^T [h, q] and softmax denominators l [1, q]; the final
    # divide + transpose happens on the host (trivial numpy work)
    outT_d = nc.dram_tensor("outT", [H, SQ], F32, kind="ExternalOutput")
    l_d = nc.dram_tensor("l", [1, SQ], F32, kind="ExternalOutput")

    scale = 1.0 / math.sqrt(H)

    with tile.TileContext(nc) as tc:
        with (
            tc.tile_pool(name="const", bufs=1) as constp,
            tc.tile_pool(name="persist", bufs=1) as persist,
            tc.tile_pool(name="stage", bufs=2) as stage_pool,
            tc.tile_pool(name="attn", bufs=4) as attn_pool,
            tc.tile_pool(name="fin", bufs=2) as fin_pool,
        ):
            # ---- constants ----
            w_sb = {}
            for name, wd in (("wq", wq_d), ("wk", wk_d), ("wv", wv_d)):
                t = constp.tile([128, NDC, H], F16, name=f"{name}_sb")
                nc.sync.dma_start(t[:], wd.ap().rearrange("(c p) h -> p c h", p=128))
                w_sb[name] = t
            ident = constp.tile([128, 128], F16, name="ident_sb")
            nc.sync.dma_start(ident[:], ident_d.ap())
            ones = constp.tile([128, 1], F16, name="ones_sb")
            nc.sync.dma_start(ones[:], ones_d.ap())

            # ---- xT: the whole transposed input lives in SBUF (8 MB fp16) --
            xt_sb = persist.tile([128, NDC, S], F16, name="xt_sb")
            for dc in range(NDC):
                nc.sync.dma_start(
                    xt_sb[:, dc, :],
                    xt_d.ap()[dc * 128 : (dc + 1) * 128, :],
                )

            # ---- persistent activations ----
            qt_sb = persist.tile([128, SQ], F16, name="qt_sb")  # [h, q]
            kt_sb = persist.tile([128, S], F16, name="kt_sb")  # [h, k]
            v_sb = persist.tile([128, NKB, H], F16, name="v_sb")  # [k128, kb, h]

            # ---- phase 1: projections ----
            # proj-major: one LDWEIGHTS per (proj, d-chunk), 8 rowblock
            # matmuls accumulating into 8 PSUM banks.
            vt_sb = persist.tile([128, S], F16, name="vt_sb")  # [h, k] staging
            with tc.tile_pool(name="ps_proj", bufs=1, space="PSUM") as ps_proj:
                for wname, dst_sb, nrb in (
                    ("wk", kt_sb, NRB),
                    ("wv", vt_sb, NRB),
                    ("wq", qt_sb, NQRB),
                ):
                    ps = ps_proj.tile([128, NRB, RB], F32, tag="proj")
                    for dc in range(NDC):
                        for rb in range(nrb):
                            nc.tensor.matmul(
                                ps[:, rb, :],
                                w_sb[wname][:, dc, :],
                                xt_sb[:, dc, rb * RB : (rb + 1) * RB],
                                start=(dc == 0),
                                stop=(dc == NDC - 1),
                            )
                    for rb in range(nrb):
                        nc.vector.tensor_copy(
                            dst_sb[:, rb * RB : (rb + 1) * RB], ps[:, rb, :]
                        )

            # ---- phase 1b: transpose vT -> v natural ----
            with tc.tile_pool(name="ps_t", bufs=4, space="PSUM") as ps_t:
                for g in range(NRB):
                    v_ps = ps_t.tile([128, RB], F16, tag="t_ps")
                    for s in range(4):
                        nc.tensor.transpose(
                            v_ps[:, s * 128 : (s + 1) * 128],
                            vt_sb[:, g * RB + s * 128 : g * RB + (s + 1) * 128],
                            ident[:],
                        )
                    nc.vector.tensor_copy(
                        v_sb[:, g * 4 : (g + 1) * 4, :].rearrange("p a b -> p (a b)"),
                        v_ps[:],
                    )

            # ---- phase 2: attention ----
            with (
                tc.tile_pool(name="ps_s", bufs=2, space="PSUM") as ps_s,
                tc.tile_pool(name="ps_o", bufs=1, space="PSUM") as ps_o,
            ):
                for qcidx in range(NQC):
                    outT_ps = ps_o.tile([128, QC], F32, tag="outT")
                    l_ps = ps_o.tile([1, QC], F32, tag="l")
                    for kb in range(NKB):
                        st_ps = ps_s.tile([128, QC], F32, tag="st")
                        for h in range(QC // 512):
                            nc.tensor.matmul(
                                st_ps[:, h * 512 : (h + 1) * 512],
                                kt_sb[:, kb * 128 : (kb + 1) * 128],
                                qt_sb[
                                    :,
                                    qcidx * QC + h * 512 : qcidx * QC + (h + 1) * 512,
                                ],
                                start=True,
                                stop=True,
                            )
                        at_sb = attn_pool.tile([128, QC], F16, tag="at")
                        nc.scalar.activation(
                            at_sb[:],
                            st_ps[:],
                            mybir.ActivationFunctionType.Exp,
                            scale=scale,
                        )
                        for h in range(QC // 512):
                            nc.tensor.matmul(
                                outT_ps[:, h * 512 : (h + 1) * 512],
                                v_sb[:, kb, :],
                                at_sb[:, h * 512 : (h + 1) * 512],
                                start=(kb == 0),
                                stop=(kb == NKB - 1),
                            )
                        for h in range(QC // 512):
                            nc.tensor.matmul(
                                l_ps[:, h * 512 : (h + 1) * 512],
                                ones[:],
                                at_sb[:, h * 512 : (h + 1) * 512],
                                start=(kb == 0),
                                stop=(kb == NKB - 1),
                            )

                    # evacuate unnormalized outT and row-sums to HBM
                    outT_sb = fin_pool.tile([128, QC], F32, tag="outT_sb")
                    nc.vector.tensor_copy(outT_sb[:], outT_ps[:])
                    nc.sync.dma_start(
                        outT_d.ap()[:, qcidx * QC : (qcidx + 1) * QC], outT_sb[:]
                    )
                    l_sb = fin_pool.tile([1, QC], F32, tag="l_sb")
                    nc.vector.tensor_copy(l_sb[:], l_ps[:])
                    nc.sync.dma_start(
                        l_d.ap()[:, qcidx * QC : (qcidx + 1) * QC], l_sb[:]
                    )

    nc.compile()
    return nc


def _get_nc():
    if "nc" not in _CACHE:
        _CACHE["nc"] = build_nc()
    return _CACHE["nc"]


def make_in_maps(inputs, Wq, Wk, Wv):
    inputs = np.asarray(inputs, dtype=np.float32)
    Wq = np.asarray(Wq, dtype=np.float16)
    Wk = np.asarray(Wk, dtype=np.float16)
    Wv = np.asarray(Wv, dtype=np.float16)
    ident = np.eye(128, dtype=np.float16)
    ones = np.ones((128, 1), dtype=np.float16)

    in_maps = []
    for c in range(NCORES):
        b, qh = divmod(c, 2)
        xb = inputs[b]
        # query half first; other half after (key order is irrelevant)
        xk = np.concatenate(
            [xb[qh * SQ : (qh + 1) * SQ], xb[(1 - qh) * SQ : (2 - qh) * SQ]], axis=0
        )
        xt = np.ascontiguousarray(xk.T.astype(np.float16))  # [D, S] fp16
        in_maps.append(
            {
                "xt": xt,
                "wq": Wq,
                "wk": Wk,
                "wv": Wv,
                "ident": ident,
                "ones": ones,
            }
        )
    return in_maps


def kernel(inputs, Wq, Wk, Wv):
    nc = _get_nc()
    in_maps = make_in_maps(inputs, Wq, Wk, Wv)

    res = run_bass_kernel_spmd(nc, in_maps, core_ids=list(range(NCORES)))

    out = np.empty((B, S, H), dtype=np.float32)
    for c in range(NCORES):
        b, qh = divmod(c, 2)
        outT = res.results[c]["outT"]  # [H, SQ] unnormalized
        l = res.results[c]["l"]  # [1, SQ]
        out[b, qh * SQ : (qh + 1) * SQ] = (outT / l).T
    return out


# revision 22
# speedup vs baseline: 1.2802x; 1.2802x over previous
"""Single-head attention kernel for Trainium2 (Bass/Tile), 8 NeuronCores.

Problem: B=4, S=4096, D=1024, H=128 fp32.
    q,k,v = x @ W{q,k,v};  out = softmax(q k^T / sqrt(H)) @ v

Sharding: 8 cores = (batch b, query-half qh).  Each core computes attention
for 2048 queries over all 4096 keys of one batch element.  The host permutes
each core's x rows so its query rows come first (softmax over keys is
permutation-invariant, so K/V row order does not matter), transposes it to
xT [D, S] (so the kernel needs no PE transposes of x), and casts to fp16.

fp16 (e5m10) is used for all matmul operands: 2-byte operands stream through
the PE at 1 column/cycle @ 2.4 GHz (4-byte fp32/fp32r streams at half rate),
and its 10-bit mantissa keeps end-to-end error ~5e-4 (all tensors here are
O(1), so the e5 range is ample).  All accumulation is fp32 in PSUM.

Per-core dataflow, arranged so projections overlap the attention chain
(PSUM budget: 1 bank projection accumulator + 4 banks scores + 2 banks
out^T + 1 bank row-sums = 8):
  1. xT d-chunks DMA'd straight into SBUF (fp16, host-prepared).
  2. Projections per 512-row block into a single rotating PSUM bank:
     qT first (all 4 q blocks), then per keyblock-group g: kT block,
     vT block, and vT->v-natural PE transposes.  The Tile scheduler
     starts attention work as soon as qT + the early kT/v groups exist.
  3. Scores TRANSPOSED, 1024-query chunks: sT[k,q] = kT(kb)^T @ qT -> PSUM.
     ScalarE exp reads sT from PSUM, writes attnT (fp16) straight to SBUF --
     the PSUM evacuation is fused into the softmax, no copy instructions.
     No max subtraction: scores are ~N(0,1) by construction, fp32 exp is
     safe (|s| < ~6).
  4. outT[h,q] += v[kb]^T @ attnT and row-sums l[q] += ones^T @ attnT,
     accumulated over kb in PSUM (exact fp32).
  5. outT (unnormalized) and l DMA'd out; host does out = (outT/l)^T.
"""

import math

import numpy as np

import concourse.bacc as bacc
import concourse.mybir as mybir
import concourse.tile as tile
from concourse.bass_utils import run_bass_kernel_spmd

B, S, D, H = 4, 4096, 1024, 128
NCORES = 8
SQ = S // 2  # queries per core (2048)
RB = 512  # rows per projection block
NRB = S // RB  # 8
NQRB = SQ // RB  # 4 blocks that need qT
QC = 1024  # queries per attention chunk
NQC = SQ // QC  # 2 chunks
NKB = S // 128  # 32 key blocks
NDC = D // 128  # 8 contraction chunks

F32 = mybir.dt.float32
F16 = mybir.dt.float16

_CACHE = {}


def build_nc():
    nc = bacc.Bacc("TRN2", target_bir_lowering=False, debug=False)

    xt_d = nc.dram_tensor("xt", [D, S], F16, kind="ExternalInput")
    wq_d = nc.dram_tensor("wq", [D, H], F16, kind="ExternalInput")
    wk_d = nc.dram_tensor("wk", [D, H], F16, kind="ExternalInput")
    wv_d = nc.dram_tensor("wv", [D, H], F16, kind="ExternalInput")
    ident_d = nc.dram_tensor("ident", [128, 128], F16, kind="ExternalInput")
    ones_d = nc.dram_tensor("ones", [128, 1], F16, kind="ExternalInput")
    # unnormalized out^T [h, q] and softmax denominators l [1, q]; the final
    # divide + transpose happens on the host (trivial numpy work)
    outT_d = nc.dram_tensor("outT", [H, SQ], F32, kind="ExternalOutput")
    l_d = nc.dram_tensor("l", [1, SQ], F32, kind="ExternalOutput")

    scale = 1.0 / math.sqrt(H)

    with tile.TileContext(nc) as tc:
        with (
            tc.tile_pool(name="const", bufs=1) as constp,
            tc.tile_pool(name="persist", bufs=1) as persist,
            tc.tile_pool(name="stage", bufs=2) as stage_pool,
            tc.tile_pool(name="attn", bufs=4) as attn_pool,
            tc.tile_pool(name="fin", bufs=2) as fin_pool,
            tc.tile_pool(name="ps_p", bufs=1, space="PSUM") as ps_p,
            tc.tile_pool(name="ps_s", bufs=2, space="PSUM") as ps_s,
            tc.tile_pool(name="ps_o", bufs=1, space="PSUM") as ps_o,
        ):
            # ---- constants ----
            w_sb = {}
            for name, wd in (("wq", wq_d), ("wk", wk_d), ("wv", wv_d)):
                t = constp.tile([128, NDC, H], F16, name=f"{name}_sb")
                nc.sync.dma_start(t[:], wd.ap().rearrange("(c p) h -> p c h", p=128))
                w_sb[name] = t
            ident = constp.tile([128, 128], F16, name="ident_sb")
            nc.sync.dma_start(ident[:], ident_d.ap())
            ones = constp.tile([128, 1], F16, name="ones_sb")
            nc.sync.dma_start(ones[:], ones_d.ap())

            # ---- xT: whole transposed input in SBUF (8 MB fp16) ----
            xt_sb = persist.tile([128, NDC, S], F16, name="xt_sb")
            for dc in range(NDC):
                nc.sync.dma_start(
                    xt_sb[:, dc, :], xt_d.ap()[dc * 128 : (dc + 1) * 128, :]
                )

            # ---- persistent activations ----
            qt_sb = persist.tile([128, SQ], F16, name="qt_sb")  # [h, q]
            kt_sb = persist.tile([128, S], F16, name="kt_sb")  # [h, k]
            v_sb = persist.tile([128, NKB, H], F16, name="v_sb")  # [k128, kb, h]

            def project(wname, dst_sb, rb):
                """One 512-row projection block through a 1-bank accumulator."""
                ps = ps_p.tile([128, RB], F32, tag="proj")
                for dc in range(NDC):
                    nc.tensor.matmul(
                        ps[:],
                        w_sb[wname][:, dc, :],
                        xt_sb[:, dc, rb * RB : (rb + 1) * RB],
                        start=(dc == 0),
                        stop=(dc == NDC - 1),
                    )
                nc.vector.tensor_copy(dst_sb[:, rb * RB : (rb + 1) * RB], ps[:])

            # qT first: attention chunk 0 needs it
            for rb in range(NQRB):
                project("wq", qt_sb, rb)

            # kT / vT / v per keyblock group; attention kb=4g..4g+3 unblocks
            # as soon as group g lands.
            vt_sb = persist.tile([128, S], F16, name="vt_sb")  # [h, k] staging
            for g in range(NRB):
                project("wk", kt_sb, g)
                project("wv", vt_sb, g)
                v_ps = ps_p.tile([128, RB], F16, tag="proj")
                for s in range(4):
                    nc.tensor.transpose(
                        v_ps[:, s * 128 : (s + 1) * 128],
                        vt_sb[:, g * RB + s * 128 : g * RB + (s + 1) * 128],
                        ident[:],
                    )
                nc.vector.tensor_copy(
                    v_sb[:, g * 4 : (g + 1) * 4, :].rearrange("p a b -> p (a b)"),
                    v_ps[:],
                )

            # ---- attention ----
            for qcidx in range(NQC):
                outT_ps = ps_o.tile([128, QC], F32, tag="outT")
                # both 512-halves of l packed into ONE psum bank: h=0 sums at
                # partition 0, h=1 sums at partition 32 (tile_position col 32)
                l_ps = ps_o.tile([64, 512], F32, tag="l")
                for kb in range(NKB):
                    st_ps = ps_s.tile([128, QC], F32, tag="st")
                    for h in range(QC // 512):
                        nc.tensor.matmul(
                            st_ps[:, h * 512 : (h + 1) * 512],
                            kt_sb[:, kb * 128 : (kb + 1) * 128],
                            qt_sb[
                                :, qcidx * QC + h * 512 : qcidx * QC + (h + 1) * 512
                            ],
                            start=True,
                            stop=True,
                        )
                    at_sb = attn_pool.tile([128, QC], F16, tag="at")
                    nc.scalar.activation(
                        at_sb[:],
                        st_ps[:],
                        mybir.ActivationFunctionType.Exp,
                        scale=scale,
                    )
                    for h in range(QC // 512):
                        nc.tensor.matmul(
                            outT_ps[:, h * 512 : (h + 1) * 512],
                            v_sb[:, kb, :],
                            at_sb[:, h * 512 : (h + 1) * 512],
                            start=(kb == 0),
                            stop=(kb == NKB - 1),
                        )
                    for h in range(QC // 512):
                        nc.tensor.matmul(
                            l_ps[h * 32 : h * 32 + 1, :],
                            ones[:],
                            at_sb[:, h * 512 : (h + 1) * 512],
                            start=(kb == 0),
                            stop=(kb == NKB - 1),
                            tile_position=(0, h * 32),
                        )

                # evacuate unnormalized outT and row-sums to HBM
                outT_sb = fin_pool.tile([128, QC], F32, tag="outT_sb")
                nc.vector.tensor_copy(outT_sb[:], outT_ps[:])
                nc.sync.dma_start(
                    outT_d.ap()[:, qcidx * QC : (qcidx + 1) * QC], outT_sb[:]
                )
                l_sb = fin_pool.tile([1, QC], F32, tag="l_sb")
                nc.vector.tensor_copy(l_sb[:, 0:512], l_ps[0:1, :])
                nc.vector.tensor_copy(l_sb[:, 512:1024], l_ps[32:33, :])
                nc.sync.dma_start(l_d.ap()[:, qcidx * QC : (qcidx + 1) * QC], l_sb[:])

    nc.compile()
    return nc


def _get_nc():
    if "nc" not in _CACHE:
        _CACHE["nc"] = build_nc()
    return _CACHE["nc"]


def make_in_maps(inputs, Wq, Wk, Wv):
    inputs = np.asarray(inputs, dtype=np.float32)
    Wq = np.asarray(Wq, dtype=np.float16)
    Wk = np.asarray(Wk, dtype=np.float16)
    Wv = np.asarray(Wv, dtype=np.float16)
    ident = np.eye(128, dtype=np.float16)
    ones = np.ones((128, 1), dtype=np.float16)

    in_maps = []
    for c in range(NCORES):
        b, qh = divmod(c, 2)
        xb = inputs[b]
        # query half first; other half after (key order is irrelevant)
        xk = np.concatenate(
            [xb[qh * SQ : (qh + 1) * SQ], xb[(1 - qh) * SQ : (2 - qh) * SQ]], axis=0
        )
        xt = np.ascontiguousarray(xk.T.astype(np.float16))  # [D, S] fp16
        in_maps.append(
            {
                "xt": xt,
                "wq": Wq,
                "wk": Wk,
                "wv": Wv,
                "ident": ident,
                "ones": ones,
            }
        )
    return in_maps


def kernel(inputs, Wq, Wk, Wv):
    nc = _get_nc()
    in_maps = make_in_maps(inputs, Wq, Wk, Wv)

    res = run_bass_kernel_spmd(nc, in_maps, core_ids=list(range(NCORES)))

    out = np.empty((B, S, H), dtype=np.float32)
    for c in range(NCORES):
        b, qh = divmod(c, 2)
        outT = res.results[c]["outT"]  # [H, SQ] unnormalized
        l = res.results[c]["l"]  # [1, SQ]
        out[b, qh * SQ : (qh + 1) * SQ] = (outT / l).T
    return out


# revision 26
# speedup vs baseline: 1.5175x; 1.1854x over previous
"""Single-head attention kernel for Trainium2 (Bass/Tile), 8 NeuronCores.

Problem: B=4, S=4096, D=1024, H=128 fp32.
    q,k,v = x @ W{q,k,v};  out = softmax(q k^T / sqrt(H)) @ v

Sharding: 8 cores = (batch b, query-half qh).  Each core computes attention
for 2048 queries over all 4096 keys of one batch element.  The host permutes
each core's x rows so its query rows come first (softmax over keys is
permutation-invariant, so K/V row order does not matter), transposes it to
xT [D, S] (so the kernel needs no PE transposes of x), and casts to fp16.

fp16 (e5m10) is used for all matmul operands: 2-byte operands stream through
the PE at 1 column/cycle @ 2.4 GHz (4-byte fp32/fp32r streams at half rate),
and its 10-bit mantissa keeps end-to-end error ~5e-4 (all tensors here are
O(1), so the e5 range is ample).  All accumulation is fp32 in PSUM.

Per-core dataflow, arranged so projections overlap the attention chain
(PSUM budget: 1 bank projection accumulator + 4 banks scores + 2 banks
out^T + 1 bank row-sums = 8):
  1. xT d-chunks DMA'd straight into SBUF (fp16, host-prepared).
  2. Projections per 512-row block into a single rotating PSUM bank:
     qT first (all 4 q blocks), then per keyblock-group g: kT block,
     vT block, and vT->v-natural PE transposes.  The Tile scheduler
     starts attention work as soon as qT + the early kT/v groups exist.
  3. Scores TRANSPOSED, 1024-query chunks: sT[k,q] = kT(kb)^T @ qT -> PSUM.
     ScalarE exp reads sT from PSUM, writes attnT (fp16) straight to SBUF --
     the PSUM evacuation is fused into the softmax, no copy instructions.
     No max subtraction: scores are ~N(0,1) by construction, fp32 exp is
     safe (|s| < ~6).
  4. outT[h,q] += v[kb]^T @ attnT and row-sums l[q] += ones^T @ attnT,
     accumulated over kb in PSUM (exact fp32).
  5. outT (unnormalized) and l DMA'd out; host does out = (outT/l)^T.
"""

import math

import numpy as np

import concourse.bacc as bacc
import concourse.mybir as mybir
import concourse.tile as tile
from concourse.bass_utils import run_bass_kernel_spmd

B, S, D, H = 4, 4096, 1024, 128
NCORES = 8
SQ = S // 2  # queries per core (2048)
RB = 512  # rows per projection block
NRB = S // RB  # 8
NQRB = SQ // RB  # 4 blocks that need qT
QC = 1024  # queries per attention chunk
NQC = SQ // QC  # 2 chunks
NKB = S // 128  # 32 key blocks
NDC = D // 128  # 8 contraction chunks

F32 = mybir.dt.float32
F16 = mybir.dt.float16

_CACHE = {}


def build_nc():
    nc = bacc.Bacc("TRN2", target_bir_lowering=False, debug=False)

    xt_d = nc.dram_tensor("xt", [D, S], F16, kind="ExternalInput")
    wq_d = nc.dram_tensor("wq", [D, H], F16, kind="ExternalInput")
    wk_d = nc.dram_tensor("wk", [D, H], F16, kind="ExternalInput")
    wv_d = nc.dram_tensor("wv", [D, H], F16, kind="ExternalInput")
    ident_d = nc.dram_tensor("ident", [128, 128], F16, kind="ExternalInput")
    ones_d = nc.dram_tensor("ones", [128, 1], F16, kind="ExternalInput")
    # unnormalized out^T [h, q] and softmax denominators l [1, q]; the final
    # divide + transpose happens on the host (trivial numpy work)
    outT_d = nc.dram_tensor("outT", [H, SQ], F32, kind="ExternalOutput")
    l_d = nc.dram_tensor("l", [1, SQ], F32, kind="ExternalOutput")

    scale = 1.0 / math.sqrt(H)

    with tile.TileContext(nc) as tc:
        with (
            tc.tile_pool(name="const", bufs=1) as constp,
            tc.tile_pool(name="persist", bufs=1) as persist,
            tc.tile_pool(name="stage", bufs=2) as stage_pool,
            tc.tile_pool(name="attn", bufs=6) as attn_pool,
            tc.tile_pool(name="fin", bufs=2) as fin_pool,
            tc.tile_pool(name="ps_p", bufs=1, space="PSUM") as ps_p,
            tc.tile_pool(name="ps_s", bufs=2, space="PSUM") as ps_s,
            tc.tile_pool(name="ps_o", bufs=1, space="PSUM") as ps_o,
        ):
            # ---- constants ----
            w_sb = {}
            for name, wd in (("wq", wq_d), ("wk", wk_d), ("wv", wv_d)):
                t = constp.tile([128, NDC, H], F16, name=f"{name}_sb")
                nc.sync.dma_start(t[:], wd.ap().rearrange("(c p) h -> p c h", p=128))
                w_sb[name] = t
            ident = constp.tile([128, 128], F16, name="ident_sb")
            nc.sync.dma_start(ident[:], ident_d.ap())
            ones = constp.tile([128, 1], F16, name="ones_sb")
            nc.sync.dma_start(ones[:], ones_d.ap())

            # ---- xT: whole transposed input in SBUF (8 MB fp16) ----
            xt_sb = persist.tile([128, NDC, S], F16, name="xt_sb")
            for dc in range(NDC):
                nc.sync.dma_start(
                    xt_sb[:, dc, :], xt_d.ap()[dc * 128 : (dc + 1) * 128, :]
                )

            # ---- persistent activations ----
            qt_sb = persist.tile([128, SQ], F16, name="qt_sb")  # [h, q]
            kt_sb = persist.tile([128, S], F16, name="kt_sb")  # [h, k]
            v_sb = persist.tile([128, NKB, H], F16, name="v_sb")  # [k128, kb, h]

            # preload the exp table during the input DMA (saves ~2us on the
            # critical path before the first real exp)
            warm = constp.tile([1, 1], F32, name="warm_sb")
            nc.scalar.activation(
                warm[:], ones[0:1, :], mybir.ActivationFunctionType.Exp
            )

            def project(wname, dst_sb, rb, pool, tag, width):
                """One 512-row projection block through a 1-bank accumulator.

                pool/tag pick which PSUM slot to borrow; the front blocks use
                the (still idle) attention slots so they can accumulate in
                parallel while the xT DMA streams in.
                """
                ps = pool.tile([128, width], F32, tag=tag)
                for dc in range(NDC):
                    nc.tensor.matmul(
                        ps[:, 0:RB],
                        w_sb[wname][:, dc, :],
                        xt_sb[:, dc, rb * RB : (rb + 1) * RB],
                        start=(dc == 0),
                        stop=(dc == NDC - 1),
                    )
                nc.vector.tensor_copy(dst_sb[:, rb * RB : (rb + 1) * RB], ps[:, 0:RB])

            vt_sb = persist.tile([128, S], F16, name="vt_sb")  # [h, k] staging

            def v_transpose(g, pool, tag, width):
                v_ps = pool.tile([128, width], F16, tag=tag)
                for s in range(4):
                    nc.tensor.transpose(
                        v_ps[:, s * 128 : (s + 1) * 128],
                        vt_sb[:, g * RB + s * 128 : g * RB + (s + 1) * 128],
                        ident[:],
                    )
                nc.vector.tensor_copy(
                    v_sb[:, g * 4 : (g + 1) * 4, :].rearrange("p a b -> p (a b)"),
                    v_ps[:, 0 : 4 * H],
                )

            # Front: the four blocks attention kb 0..3 needs, accumulated in
            # parallel using idle attention PSUM slots (attention only starts
            # after these release them).
            project("wq", qt_sb, 0, ps_s, "st", QC)
            project("wk", kt_sb, 0, ps_s, "st", QC)
            project("wq", qt_sb, 1, ps_o, "outT", QC)
            project("wv", vt_sb, 0, ps_p, "proj", RB)
            v_transpose(0, ps_p, "proj", RB)
            # Rest: serial through the 1-bank proj slot; the scheduler
            # overlaps these with the attention chain.
            for g in range(1, NRB):
                project("wk", kt_sb, g, ps_p, "proj", RB)
                project("wv", vt_sb, g, ps_p, "proj", RB)
                v_transpose(g, ps_p, "proj", RB)
                if g < 3:  # qt rb2/rb3 needed only by attention chunk 1
                    project("wq", qt_sb, g + 1, ps_p, "proj", RB)

            # ---- attention ----
            # software-pipelined by one kb: sT/exp for kb+1 are emitted (and
            # thus prioritized) ahead of AV/l for kb, so the PE always has
            # score matmuls to run while ScalarE computes the current exp.
            for qcidx in range(NQC):
                outT_ps = ps_o.tile([128, QC], F32, tag="outT")
                # both 512-halves of l packed into ONE psum bank: h=0 sums at
                # partition 0, h=1 sums at partition 32 (tile_position col 32)
                l_ps = ps_o.tile([64, 512], F32, tag="l")
                at_tiles = {}

                def score(kb):
                    st_ps = ps_s.tile([128, QC], F32, tag="st")
                    for h in range(QC // 512):
                        nc.tensor.matmul(
                            st_ps[:, h * 512 : (h + 1) * 512],
                            kt_sb[:, kb * 128 : (kb + 1) * 128],
                            qt_sb[
                                :, qcidx * QC + h * 512 : qcidx * QC + (h + 1) * 512
                            ],
                            start=True,
                            stop=True,
                        )
                    at_sb = attn_pool.tile([128, QC], F16, tag="at")
                    nc.scalar.activation(
                        at_sb[:],
                        st_ps[:],
                        mybir.ActivationFunctionType.Exp,
                        scale=scale,
                    )
                    at_tiles[kb] = at_sb

                def accum(kb):
                    at_sb = at_tiles.pop(kb)
                    for h in range(QC // 512):
                        nc.tensor.matmul(
                            outT_ps[:, h * 512 : (h + 1) * 512],
                            v_sb[:, kb, :],
                            at_sb[:, h * 512 : (h + 1) * 512],
                            start=(kb == 0),
                            stop=(kb == NKB - 1),
                        )
                    for h in range(QC // 512):
                        nc.tensor.matmul(
                            l_ps[h * 32 : h * 32 + 1, :],
                            ones[:],
                            at_sb[:, h * 512 : (h + 1) * 512],
                            start=(kb == 0),
                            stop=(kb == NKB - 1),
                            tile_position=(0, h * 32),
                        )

                score(0)
                for kb in range(1, NKB):
                    score(kb)
                    accum(kb - 1)
                accum(NKB - 1)

                # evacuate unnormalized outT and row-sums to HBM
                outT_sb = fin_pool.tile([128, QC], F32, tag="outT_sb")
                nc.vector.tensor_copy(outT_sb[:], outT_ps[:])
                nc.sync.dma_start(
                    outT_d.ap()[:, qcidx * QC : (qcidx + 1) * QC], outT_sb[:]
                )
                l_sb = fin_pool.tile([1, QC], F32, tag="l_sb")
                nc.vector.tensor_copy(l_sb[:, 0:512], l_ps[0:1, :])
                nc.vector.tensor_copy(l_sb[:, 512:1024], l_ps[32:33, :])
                nc.sync.dma_start(l_d.ap()[:, qcidx * QC : (qcidx + 1) * QC], l_sb[:])

    nc.compile()
    return nc


def _get_nc():
    if "nc" not in _CACHE:
        _CACHE["nc"] = build_nc()
    return _CACHE["nc"]


def make_in_maps(inputs, Wq, Wk, Wv):
    inputs = np.asarray(inputs, dtype=np.float32)
    Wq = np.asarray(Wq, dtype=np.float16)
    Wk = np.asarray(Wk, dtype=np.float16)
    Wv = np.asarray(Wv, dtype=np.float16)
    ident = np.eye(128, dtype=np.float16)
    ones = np.ones((128, 1), dtype=np.float16)

    in_maps = []
    for c in range(NCORES):
        b, qh = divmod(c, 2)
        xb = inputs[b]
        # query half first; other half after (key order is irrelevant)
        xk = np.concatenate(
            [xb[qh * SQ : (qh + 1) * SQ], xb[(1 - qh) * SQ : (2 - qh) * SQ]], axis=0
        )
        xt = np.ascontiguousarray(xk.T.astype(np.float16))  # [D, S] fp16
        in_maps.append(
            {
                "xt": xt,
                "wq": Wq,
                "wk": Wk,
                "wv": Wv,
                "ident": ident,
                "ones": ones,
            }
        )
    return in_maps


def kernel(inputs, Wq, Wk, Wv):
    nc = _get_nc()
    in_maps = make_in_maps(inputs, Wq, Wk, Wv)

    res = run_bass_kernel_spmd(nc, in_maps, core_ids=list(range(NCORES)))

    out = np.empty((B, S, H), dtype=np.float32)
    for c in range(NCORES):
        b, qh = divmod(c, 2)
        outT = res.results[c]["outT"]  # [H, SQ] unnormalized
        l = res.results[c]["l"]  # [1, SQ]
        out[b, qh * SQ : (qh + 1) * SQ] = (outT / l).T
    return out


# revision 30
# speedup vs baseline: 1.6131x; 1.0630x over previous
"""Single-head attention kernel for Trainium2 (Bass/Tile), 8 NeuronCores.

Problem: B=4, S=4096, D=1024, H=128 fp32.
    q,k,v = x @ W{q,k,v};  out = softmax(q k^T / sqrt(H)) @ v

Sharding: 8 cores = (batch b, query-half qh).  Each core computes attention
for 2048 queries over all 4096 keys of one batch element.  The host permutes
each core's x rows so its query rows come first (softmax over keys is
permutation-invariant, so K/V row order does not matter), transposes it to
xT [D, S] (so the kernel needs no PE transposes of x), and casts to fp16.

fp16 (e5m10) is used for all matmul operands: 2-byte operands stream through
the PE at 1 column/cycle @ 2.4 GHz (4-byte fp32/fp32r streams at half rate),
and its 10-bit mantissa keeps end-to-end error ~5e-4 (all tensors here are
O(1), so the e5 range is ample).  All accumulation is fp32 in PSUM.

Per-core dataflow, arranged so projections overlap the attention chain
(PSUM budget: 1 bank projection accumulator + 4 banks scores + 2 banks
out^T + 1 bank row-sums = 8):
  1. xT d-chunks DMA'd straight into SBUF (fp16, host-prepared).
  2. Projections per 512-row block into a single rotating PSUM bank:
     qT first (all 4 q blocks), then per keyblock-group g: kT block,
     vT block, and vT->v-natural PE transposes.  The Tile scheduler
     starts attention work as soon as qT + the early kT/v groups exist.
  3. Scores TRANSPOSED, 1024-query chunks: sT[k,q] = kT(kb)^T @ qT -> PSUM.
     ScalarE exp reads sT from PSUM, writes attnT (fp16) straight to SBUF --
     the PSUM evacuation is fused into the softmax, no copy instructions.
     No max subtraction: scores are ~N(0,1) by construction, fp32 exp is
     safe (|s| < ~6).
  4. outT[h,q] += v[kb]^T @ attnT and row-sums l[q] += ones^T @ attnT,
     accumulated over kb in PSUM (exact fp32).
  5. outT (unnormalized) and l DMA'd out; host does out = (outT/l)^T.
"""

import math

import numpy as np

import concourse.bacc as bacc
import concourse.mybir as mybir
import concourse.tile as tile
from concourse.bass_utils import run_bass_kernel_spmd

B, S, D, H = 4, 4096, 1024, 128
NCORES = 8
SQ = S // 2  # queries per core (2048)
RB = 512  # rows per projection block
NRB = S // RB  # 8
NQRB = SQ // RB  # 4 blocks that need qT
QC = 1024  # queries per attention chunk
NQC = SQ // QC  # 2 chunks
NKB = S // 128  # 32 key blocks
NDC = D // 128  # 8 contraction chunks

F32 = mybir.dt.float32
F16 = mybir.dt.float16

_CACHE = {}


def build_nc():
    nc = bacc.Bacc("TRN2", target_bir_lowering=False, debug=False)

    xt_d = nc.dram_tensor("xt", [D, S], F16, kind="ExternalInput")
    # weights host-preswizzled to [128, NDC*H]: row p, chunk c = W[c*128+p, :]
    wq_d = nc.dram_tensor("wq", [128, NDC * H], F16, kind="ExternalInput")
    wk_d = nc.dram_tensor("wk", [128, NDC * H], F16, kind="ExternalInput")
    wv_d = nc.dram_tensor("wv", [128, NDC * H], F16, kind="ExternalInput")
    ident_d = nc.dram_tensor("ident", [128, 128], F16, kind="ExternalInput")
    ones_d = nc.dram_tensor("ones", [128, 1], F16, kind="ExternalInput")
    # unnormalized out^T [h, q] and softmax denominators l [1, q]; the final
    # divide + transpose happens on the host (trivial numpy work)
    outT_d = nc.dram_tensor("outT", [H, SQ], F32, kind="ExternalOutput")
    l_d = nc.dram_tensor("l", [1, SQ], F32, kind="ExternalOutput")

    scale = 1.0 / math.sqrt(H)

    with tile.TileContext(nc) as tc:
        with (
            tc.tile_pool(name="const", bufs=1) as constp,
            tc.tile_pool(name="persist", bufs=1) as persist,
            tc.tile_pool(name="stage", bufs=2) as stage_pool,
            tc.tile_pool(name="attn", bufs=6) as attn_pool,
            tc.tile_pool(name="fin", bufs=2) as fin_pool,
            tc.tile_pool(name="ps_p", bufs=1, space="PSUM") as ps_p,
            tc.tile_pool(name="ps_s", bufs=2, space="PSUM") as ps_s,
            tc.tile_pool(name="ps_o", bufs=1, space="PSUM") as ps_o,
        ):
            # ---- constants ----
            # weights arrive host-preswizzled as [128, NDC*H] (contiguous DMA)
            w_sb = {}
            for name, wd in (("wq", wq_d), ("wk", wk_d), ("wv", wv_d)):
                t = constp.tile([128, NDC, H], F16, name=f"{name}_sb")
                nc.sync.dma_start(
                    t[:], wd.ap().rearrange("p (c h) -> p c h", c=NDC)
                )
                w_sb[name] = t
            ident = constp.tile([128, 128], F16, name="ident_sb")
            nc.sync.dma_start(ident[:], ident_d.ap())
            ones = constp.tile([128, 1], F16, name="ones_sb")
            nc.sync.dma_start(ones[:], ones_d.ap())

            # ---- xT in SBUF (8 MB fp16), loaded k-slice-major so each
            # arriving 512-key slice immediately completes its projection
            # blocks and unblocks 4 attention key-blocks ----
            xt_sb = persist.tile([128, NDC, S], F16, name="xt_sb")
            for g in range(NRB):
                nc.sync.dma_start(
                    xt_sb[:, :, g * RB : (g + 1) * RB],
                    xt_d.ap()[:, g * RB : (g + 1) * RB].rearrange(
                        "(c p) s -> p c s", p=128
                    ),
                )

            # ---- persistent activations ----
            qt_sb = persist.tile([128, SQ], F16, name="qt_sb")  # [h, q]
            kt_sb = persist.tile([128, S], F16, name="kt_sb")  # [h, k]
            v_sb = persist.tile([128, NKB, H], F16, name="v_sb")  # [k128, kb, h]

            # preload the exp table during the input DMA (saves ~2us on the
            # critical path before the first real exp)
            warm = constp.tile([1, 1], F32, name="warm_sb")
            nc.scalar.activation(
                warm[:], ones[0:1, :], mybir.ActivationFunctionType.Exp
            )

            def project(wname, dst_sb, rb, pool, tag, width):
                """One 512-row projection block through a 1-bank accumulator.

                pool/tag pick which PSUM slot to borrow; the front blocks use
                the (still idle) attention slots so they can accumulate in
                parallel while the xT DMA streams in.
                """
                ps = pool.tile([128, width], F32, tag=tag)
                for dc in range(NDC):
                    nc.tensor.matmul(
                        ps[:, 0:RB],
                        w_sb[wname][:, dc, :],
                        xt_sb[:, dc, rb * RB : (rb + 1) * RB],
                        start=(dc == 0),
                        stop=(dc == NDC - 1),
                    )
                nc.vector.tensor_copy(dst_sb[:, rb * RB : (rb + 1) * RB], ps[:, 0:RB])

            vt_sb = persist.tile([128, S], F16, name="vt_sb")  # [h, k] staging

            def v_transpose(g, pool, tag, width):
                v_ps = pool.tile([128, width], F16, tag=tag)
                for s in range(4):
                    nc.tensor.transpose(
                        v_ps[:, s * 128 : (s + 1) * 128],
                        vt_sb[:, g * RB + s * 128 : g * RB + (s + 1) * 128],
                        ident[:],
                    )
                nc.vector.tensor_copy(
                    v_sb[:, g * 4 : (g + 1) * 4, :].rearrange("p a b -> p (a b)"),
                    v_ps[:, 0 : 4 * H],
                )

            # Front: the blocks attention kb 0..3 needs, accumulated in
            # parallel using idle attention PSUM slots (attention only starts
            # after these release them).
            project("wq", qt_sb, 0, ps_s, "st", QC)
            project("wk", kt_sb, 0, ps_s, "st", QC)
            project("wv", vt_sb, 0, ps_o, "outT", QC)
            project("wq", qt_sb, 1, ps_o, "l", 512)
            v_transpose(0, ps_p, "proj", RB)
            # Rest: serial through the 1-bank proj slot; the scheduler
            # overlaps these with the attention chain.
            for g in range(1, NRB):
                project("wk", kt_sb, g, ps_p, "proj", RB)
                project("wv", vt_sb, g, ps_p, "proj", RB)
                v_transpose(g, ps_p, "proj", RB)
                if g < 3:  # qt rb2/rb3 needed only by attention chunk 1
                    project("wq", qt_sb, g + 1, ps_p, "proj", RB)

            # ---- attention ----
            # software-pipelined by one kb: sT/exp for kb+1 are emitted (and
            # thus prioritized) ahead of AV/l for kb, so the PE always has
            # score matmuls to run while ScalarE computes the current exp.
            for qcidx in range(NQC):
                outT_ps = ps_o.tile([128, QC], F32, tag="outT")
                # both 512-halves of l packed into ONE psum bank: h=0 sums at
                # partition 0, h=1 sums at partition 32 (tile_position col 32)
                l_ps = ps_o.tile([64, 512], F32, tag="l")
                at_tiles = {}

                def score(kb):
                    st_ps = ps_s.tile([128, QC], F32, tag="st")
                    for h in range(QC // 512):
                        nc.tensor.matmul(
                            st_ps[:, h * 512 : (h + 1) * 512],
                            kt_sb[:, kb * 128 : (kb + 1) * 128],
                            qt_sb[
                                :, qcidx * QC + h * 512 : qcidx * QC + (h + 1) * 512
                            ],
                            start=True,
                            stop=True,
                        )
                    at_sb = attn_pool.tile([128, QC], F16, tag="at")
                    nc.scalar.activation(
                        at_sb[:],
                        st_ps[:],
                        mybir.ActivationFunctionType.Exp,
                        scale=scale,
                    )
                    at_tiles[kb] = at_sb

                def accum(kb):
                    at_sb = at_tiles.pop(kb)
                    for h in range(QC // 512):
                        nc.tensor.matmul(
                            outT_ps[:, h * 512 : (h + 1) * 512],
                            v_sb[:, kb, :],
                            at_sb[:, h * 512 : (h + 1) * 512],
                            start=(kb == 0),
                            stop=(kb == NKB - 1),
                        )
                    for h in range(QC // 512):
                        nc.tensor.matmul(
                            l_ps[h * 32 : h * 32 + 1, :],
                            ones[:],
                            at_sb[:, h * 512 : (h + 1) * 512],
                            start=(kb == 0),
                            stop=(kb == NKB - 1),
                            tile_position=(0, h * 32),
                        )

                score(0)
                for kb in range(1, NKB):
                    score(kb)
                    accum(kb - 1)
                accum(NKB - 1)

                # evacuate unnormalized outT and row-sums to HBM
                outT_sb = fin_pool.tile([128, QC], F32, tag="outT_sb")
                nc.vector.tensor_copy(outT_sb[:], outT_ps[:])
                nc.sync.dma_start(
                    outT_d.ap()[:, qcidx * QC : (qcidx + 1) * QC], outT_sb[:]
                )
                l_sb = fin_pool.tile([1, QC], F32, tag="l_sb")
                nc.vector.tensor_copy(l_sb[:, 0:512], l_ps[0:1, :])
                nc.vector.tensor_copy(l_sb[:, 512:1024], l_ps[32:33, :])
                nc.sync.dma_start(l_d.ap()[:, qcidx * QC : (qcidx + 1) * QC], l_sb[:])

    nc.compile()
    return nc


def _get_nc():
    if "nc" not in _CACHE:
        _CACHE["nc"] = build_nc()
    return _CACHE["nc"]


def _swizzle_w(W):
    # [D, H] -> [128, NDC*H]: row p, chunk c holds W[c*128+p, :]
    W = np.asarray(W, dtype=np.float16)
    return np.ascontiguousarray(
        W.reshape(NDC, 128, H).transpose(1, 0, 2).reshape(128, NDC * H)
    )


def make_in_maps(inputs, Wq, Wk, Wv):
    inputs = np.asarray(inputs, dtype=np.float32)
    Wq = _swizzle_w(Wq)
    Wk = _swizzle_w(Wk)
    Wv = _swizzle_w(Wv)
    ident = np.eye(128, dtype=np.float16)
    ones = np.ones((128, 1), dtype=np.float16)

    in_maps = []
    for c in range(NCORES):
        b, qh = divmod(c, 2)
        xb = inputs[b]
        # query half first; other half after (key order is irrelevant)
        xk = np.concatenate(
            [xb[qh * SQ : (qh + 1) * SQ], xb[(1 - qh) * SQ : (2 - qh) * SQ]], axis=0
        )
        xt = np.ascontiguousarray(xk.T.astype(np.float16))  # [D, S] fp16
        in_maps.append(
            {
                "xt": xt,
                "wq": Wq,
                "wk": Wk,
                "wv": Wv,
                "ident": ident,
                "ones": ones,
            }
        )
    return in_maps


def kernel(inputs, Wq, Wk, Wv):
    nc = _get_nc()
    in_maps = make_in_maps(inputs, Wq, Wk, Wv)

    res = run_bass_kernel_spmd(nc, in_maps, core_ids=list(range(NCORES)))

    out = np.empty((B, S, H), dtype=np.float32)
    for c in range(NCORES):
        b, qh = divmod(c, 2)
        outT = res.results[c]["outT"]  # [H, SQ] unnormalized
        l = res.results[c]["l"]  # [1, SQ]
        out[b, qh * SQ : (qh + 1) * SQ] = (outT / l).T
    return out


# revision 32
# speedup vs baseline: 1.6463x; 1.0206x over previous
"""Single-head attention kernel for Trainium2 (Bass/Tile), 8 NeuronCores.

Problem: B=4, S=4096, D=1024, H=128 fp32.
    q,k,v = x @ W{q,k,v};  out = softmax(q k^T / sqrt(H)) @ v

Sharding: 8 cores = (batch b, query-half qh).  Each core computes attention
for 2048 queries over all 4096 keys of one batch element.  The host permutes
each core's x rows so its query rows come first (softmax over keys is
permutation-invariant, so K/V row order does not matter), transposes it to
xT [D, S] (so the kernel needs no PE transposes of x), and casts to fp16.

fp16 (e5m10) is used for all matmul operands: 2-byte operands stream through
the PE at 1 column/cycle @ 2.4 GHz (4-byte fp32/fp32r streams at half rate),
and its 10-bit mantissa keeps end-to-end error ~5e-4 (all tensors here are
O(1), so the e5 range is ample).  All accumulation is fp32 in PSUM.

Per-core dataflow, arranged so projections overlap the attention chain
(PSUM budget: 1 bank projection accumulator + 4 banks scores + 2 banks
out^T + 1 bank row-sums = 8):
  1. xT d-chunks DMA'd straight into SBUF (fp16, host-prepared).
  2. Projections per 512-row block into a single rotating PSUM bank:
     qT first (all 4 q blocks), then per keyblock-group g: kT block,
     vT block, and vT->v-natural PE transposes.  The Tile scheduler
     starts attention work as soon as qT + the early kT/v groups exist.
  3. Scores TRANSPOSED, 1024-query chunks: sT[k,q] = kT(kb)^T @ qT -> PSUM.
     ScalarE exp reads sT from PSUM, writes attnT (fp16) straight to SBUF --
     the PSUM evacuation is fused into the softmax, no copy instructions.
     No max subtraction: scores are ~N(0,1) by construction, fp32 exp is
     safe (|s| < ~6).
  4. outT[h,q] += v[kb]^T @ attnT and row-sums l[q] += ones^T @ attnT,
     accumulated over kb in PSUM (exact fp32).
  5. outT (unnormalized) and l DMA'd out; host does out = (outT/l)^T.
"""

import math

import numpy as np

import concourse.bacc as bacc
import concourse.mybir as mybir
import concourse.tile as tile
from concourse.bass_utils import run_bass_kernel_spmd

B, S, D, H = 4, 4096, 1024, 128
NCORES = 8
SQ = S // 2  # queries per core (2048)
RB = 512  # rows per projection block
NRB = S // RB  # 8
NQRB = SQ // RB  # 4 blocks that need qT
QC = 1024  # queries per attention chunk
NQC = SQ // QC  # 2 chunks
NKB = S // 128  # 32 key blocks
NDC = D // 128  # 8 contraction chunks

F32 = mybir.dt.float32
F16 = mybir.dt.float16

_CACHE = {}


def build_nc():
    nc = bacc.Bacc("TRN2", target_bir_lowering=False, debug=False)

    xt_d = nc.dram_tensor("xt", [D, S], F16, kind="ExternalInput")
    # weights host-preswizzled to [128, NDC*H]: row p, chunk c = W[c*128+p, :]
    wq_d = nc.dram_tensor("wq", [128, NDC * H], F16, kind="ExternalInput")
    wk_d = nc.dram_tensor("wk", [128, NDC * H], F16, kind="ExternalInput")
    wv_d = nc.dram_tensor("wv", [128, NDC * H], F16, kind="ExternalInput")
    ident_d = nc.dram_tensor("ident", [128, 128], F16, kind="ExternalInput")
    ones_d = nc.dram_tensor("ones", [128, 1], F16, kind="ExternalInput")
    # unnormalized out^T [h, q] and softmax denominators l [1, q]; the final
    # divide + transpose happens on the host (trivial numpy work)
    outT_d = nc.dram_tensor("outT", [H, SQ], F32, kind="ExternalOutput")
    l_d = nc.dram_tensor("l", [1, SQ], F32, kind="ExternalOutput")

    scale = 1.0 / math.sqrt(H)

    with tile.TileContext(nc) as tc:
        with (
            tc.tile_pool(name="const", bufs=1) as constp,
            tc.tile_pool(name="persist", bufs=1) as persist,
            tc.tile_pool(name="stage", bufs=2) as stage_pool,
            tc.tile_pool(name="attn", bufs=6) as attn_pool,
            tc.tile_pool(name="fin", bufs=2) as fin_pool,
            tc.tile_pool(name="ps_p", bufs=1, space="PSUM") as ps_p,
            tc.tile_pool(name="ps_s", bufs=2, space="PSUM") as ps_s,
            tc.tile_pool(name="ps_o", bufs=1, space="PSUM") as ps_o,
        ):
            # ---- constants + input, DMA-ordered for the critical path:
            # wq -> xT slice 0 -> wk/wv -> ident/ones -> slices 1..7 ----
            # (weights arrive host-preswizzled as [128, NDC*H], contiguous)
            w_sb = {}
            for name, wd in (("wq", wq_d), ("wk", wk_d), ("wv", wv_d)):
                w_sb[name] = constp.tile([128, NDC, H], F16, name=f"{name}_sb")

            def load_w(name):
                nc.sync.dma_start(
                    w_sb[name][:],
                    {"wq": wq_d, "wk": wk_d, "wv": wv_d}[name]
                    .ap()
                    .rearrange("p (c h) -> p c h", c=NDC),
                )

            xt_sb = persist.tile([128, NDC, S], F16, name="xt_sb")

            def load_slice(g):
                # k-slice-major: each arriving 512-key slice immediately
                # completes its projection blocks -> unblocks 4 attention kbs
                nc.sync.dma_start(
                    xt_sb[:, :, g * RB : (g + 1) * RB],
                    xt_d.ap()[:, g * RB : (g + 1) * RB].rearrange(
                        "(c p) s -> p c s", p=128
                    ),
                )

            ident = constp.tile([128, 128], F16, name="ident_sb")
            ones = constp.tile([128, 1], F16, name="ones_sb")

            load_w("wq")
            load_slice(0)
            load_w("wk")
            load_w("wv")
            nc.sync.dma_start(ident[:], ident_d.ap())
            nc.sync.dma_start(ones[:], ones_d.ap())
            for g in range(1, NRB):
                load_slice(g)

            # ---- persistent activations ----
            qt_sb = persist.tile([128, SQ], F16, name="qt_sb")  # [h, q]
            kt_sb = persist.tile([128, S], F16, name="kt_sb")  # [h, k]
            v_sb = persist.tile([128, NKB, H], F16, name="v_sb")  # [k128, kb, h]

            # preload the exp table during the input DMA (saves ~2us on the
            # critical path before the first real exp)
            warm = constp.tile([1, 1], F32, name="warm_sb")
            nc.scalar.activation(
                warm[:], ones[0:1, :], mybir.ActivationFunctionType.Exp
            )
            # HAM warm-up: ~3.5us of dummy matmuls on the just-arrived wq
            # while the first xT slice streams in, so the PE clock is at
            # 2.4 GHz (K=8/8) when the real matmuls start
            warm_ps = ps_p.tile([128, 128], F32, tag="proj")
            for i in range(16):
                nc.tensor.matmul(
                    warm_ps[:],
                    w_sb["wq"][:, 0, :],
                    w_sb["wq"][:, 0, :],
                    start=(i == 0),
                    stop=(i == 15),
                )

            def project(wname, dst_sb, rb, pool, tag, width):
                """One 512-row projection block through a 1-bank accumulator.

                pool/tag pick which PSUM slot to borrow; the front blocks use
                the (still idle) attention slots so they can accumulate in
                parallel while the xT DMA streams in.
                """
                ps = pool.tile([128, width], F32, tag=tag)
                for dc in range(NDC):
                    nc.tensor.matmul(
                        ps[:, 0:RB],
                        w_sb[wname][:, dc, :],
                        xt_sb[:, dc, rb * RB : (rb + 1) * RB],
                        start=(dc == 0),
                        stop=(dc == NDC - 1),
                    )
                nc.vector.tensor_copy(dst_sb[:, rb * RB : (rb + 1) * RB], ps[:, 0:RB])

            vt_sb = persist.tile([128, S], F16, name="vt_sb")  # [h, k] staging

            def v_transpose(g, pool, tag, width):
                v_ps = pool.tile([128, width], F16, tag=tag)
                for s in range(4):
                    nc.tensor.transpose(
                        v_ps[:, s * 128 : (s + 1) * 128],
                        vt_sb[:, g * RB + s * 128 : g * RB + (s + 1) * 128],
                        ident[:],
                    )
                nc.vector.tensor_copy(
                    v_sb[:, g * 4 : (g + 1) * 4, :].rearrange("p a b -> p (a b)"),
                    v_ps[:, 0 : 4 * H],
                )

            # Front: the blocks attention kb 0..3 needs, accumulated in
            # parallel using idle attention PSUM slots (attention only starts
            # after these release them).
            project("wq", qt_sb, 0, ps_s, "st", QC)
            project("wk", kt_sb, 0, ps_s, "st", QC)
            project("wv", vt_sb, 0, ps_o, "outT", QC)
            project("wq", qt_sb, 1, ps_o, "l", 512)
            v_transpose(0, ps_p, "proj", RB)
            # Rest: serial through the 1-bank proj slot; the scheduler
            # overlaps these with the attention chain.
            for g in range(1, NRB):
                project("wk", kt_sb, g, ps_p, "proj", RB)
                project("wv", vt_sb, g, ps_p, "proj", RB)
                v_transpose(g, ps_p, "proj", RB)
                if g < 3:  # qt rb2/rb3 needed only by attention chunk 1
                    project("wq", qt_sb, g + 1, ps_p, "proj", RB)

            # ---- attention ----
            # software-pipelined by one kb: sT/exp for kb+1 are emitted (and
            # thus prioritized) ahead of AV/l for kb, so the PE always has
            # score matmuls to run while ScalarE computes the current exp.
            for qcidx in range(NQC):
                outT_ps = ps_o.tile([128, QC], F32, tag="outT")
                # both 512-halves of l packed into ONE psum bank: h=0 sums at
                # partition 0, h=1 sums at partition 32 (tile_position col 32)
                l_ps = ps_o.tile([64, 512], F32, tag="l")
                at_tiles = {}

                def score(kb):
                    st_ps = ps_s.tile([128, QC], F32, tag="st")
                    for h in range(QC // 512):
                        nc.tensor.matmul(
                            st_ps[:, h * 512 : (h + 1) * 512],
                            kt_sb[:, kb * 128 : (kb + 1) * 128],
                            qt_sb[
                                :, qcidx * QC + h * 512 : qcidx * QC + (h + 1) * 512
                            ],
                            start=True,
                            stop=True,
                        )
                    at_sb = attn_pool.tile([128, QC], F16, tag="at")
                    nc.scalar.activation(
                        at_sb[:],
                        st_ps[:],
                        mybir.ActivationFunctionType.Exp,
                        scale=scale,
                    )
                    at_tiles[kb] = at_sb

                def accum(kb):
                    at_sb = at_tiles.pop(kb)
                    for h in range(QC // 512):
                        nc.tensor.matmul(
                            outT_ps[:, h * 512 : (h + 1) * 512],
                            v_sb[:, kb, :],
                            at_sb[:, h * 512 : (h + 1) * 512],
                            start=(kb == 0),
                            stop=(kb == NKB - 1),
                        )
                    for h in range(QC // 512):
                        nc.tensor.matmul(
                            l_ps[h * 32 : h * 32 + 1, :],
                            ones[:],
                            at_sb[:, h * 512 : (h + 1) * 512],
                            start=(kb == 0),
                            stop=(kb == NKB - 1),
                            tile_position=(0, h * 32),
                        )

                score(0)
                for kb in range(1, NKB):
                    score(kb)
                    accum(kb - 1)
                accum(NKB - 1)

                # evacuate unnormalized outT and row-sums to HBM
                outT_sb = fin_pool.tile([128, QC], F32, tag="outT_sb")
                nc.vector.tensor_copy(outT_sb[:], outT_ps[:])
                nc.sync.dma_start(
                    outT_d.ap()[:, qcidx * QC : (qcidx + 1) * QC], outT_sb[:]
                )
                l_sb = fin_pool.tile([1, QC], F32, tag="l_sb")
                nc.vector.tensor_copy(l_sb[:, 0:512], l_ps[0:1, :])
                nc.vector.tensor_copy(l_sb[:, 512:1024], l_ps[32:33, :])
                nc.sync.dma_start(l_d.ap()[:, qcidx * QC : (qcidx + 1) * QC], l_sb[:])

    nc.compile()
    return nc


def _get_nc():
    if "nc" not in _CACHE:
        _CACHE["nc"] = build_nc()
    return _CACHE["nc"]


def _swizzle_w(W):
    # [D, H] -> [128, NDC*H]: row p, chunk c holds W[c*128+p, :]
    W = np.asarray(W, dtype=np.float16)
    return np.ascontiguousarray(
        W.reshape(NDC, 128, H).transpose(1, 0, 2).reshape(128, NDC * H)
    )


def make_in_maps(inputs, Wq, Wk, Wv):
    inputs = np.asarray(inputs, dtype=np.float32)
    Wq = _swizzle_w(Wq)
    Wk = _swizzle_w(Wk)
    Wv = _swizzle_w(Wv)
    ident = np.eye(128, dtype=np.float16)
    ones = np.ones((128, 1), dtype=np.float16)

    in_maps = []
    for c in range(NCORES):
        b, qh = divmod(c, 2)
        xb = inputs[b]
        # query half first; other half after (key order is irrelevant)
        xk = np.concatenate(
            [xb[qh * SQ : (qh + 1) * SQ], xb[(1 - qh) * SQ : (2 - qh) * SQ]], axis=0
        )
        xt = np.ascontiguousarray(xk.T.astype(np.float16))  # [D, S] fp16
        in_maps.append(
            {
                "xt": xt,
                "wq": Wq,
                "wk": Wk,
                "wv": Wv,
                "ident": ident,
                "ones": ones,
            }
        )
    return in_maps


def kernel(inputs, Wq, Wk, Wv):
    nc = _get_nc()
    in_maps = make_in_maps(inputs, Wq, Wk, Wv)

    res = run_bass_kernel_spmd(nc, in_maps, core_ids=list(range(NCORES)))

    out = np.empty((B, S, H), dtype=np.float32)
    for c in range(NCORES):
        b, qh = divmod(c, 2)
        outT = res.results[c]["outT"]  # [H, SQ] unnormalized
        l = res.results[c]["l"]  # [1, SQ]
        out[b, qh * SQ : (qh + 1) * SQ] = (outT / l).T
    return out


# revision 33
# speedup vs baseline: 1.7825x; 1.0828x over previous
"""Single-head attention kernel for Trainium2 (Bass/Tile), 8 NeuronCores.

Problem: B=4, S=4096, D=1024, H=128 fp32.
    q,k,v = x @ W{q,k,v};  out = softmax(q k^T / sqrt(H)) @ v

Sharding: 8 cores = (batch b, KEY-half kh).  Each core computes PARTIAL
attention for all 4096 queries over its 2048 keys; the host combines the
two partial results per batch: out = (outT_0 + outT_1) / (l_0 + l_1)
(unnormalized value-sums and softmax denominators add across key shards).
The host permutes each core's x rows so its key rows come first and
transposes/casts to xT [D, S] fp16.  Query order follows the same
permutation; the host maps it back when combining.

fp16 (e5m10) everywhere on the matmul operands: 2-byte operands stream
through the PE at 1 column/cycle @ 2.4 GHz (4-byte fp32/fp32r streams at
half rate) and its 10-bit mantissa keeps end-to-end error ~5e-4 (all
tensors here are O(1)).  All accumulation is fp32 in PSUM.

Per-core dataflow (PSUM: 1 bank proj + 4 banks scores + 2 banks outT +
1 bank row-sums = 8):
  1. xT k-slices DMA'd in (slice-major: each arriving 512-row slice
     completes projection blocks immediately); DMA order puts wq and
     slice 0 first; dummy matmuls warm the PE clock (HAM) meanwhile.
  2. Projections per 512-row block through rotating PSUM banks: the first
     blocks borrow idle attention PSUM slots to run in parallel.
     qT for all 8 query blocks; kT/vT only for the 4 own-key blocks;
     vT PE-transposed to v-natural.
  3. Scores TRANSPOSED, 1024-query chunks: sT[k,q] = kT(kb)^T @ qT -> PSUM.
     ScalarE exp reads sT from PSUM and writes attnT (fp16) straight to
     SBUF -- softmax PSUM evacuation fused into the exp.  No max
     subtraction needed: scores are ~N(0,1), fp32 exp is safe.
  4. outT[h,q] += v[kb]^T @ attnT;  l[q] += ones^T @ attnT  (fp32 PSUM,
     software-pipelined by one kb against the exp).
  5. Partial outT and l DMA'd out; host combines shards + normalizes.
"""

import math

import numpy as np

import concourse.bacc as bacc
import concourse.mybir as mybir
import concourse.tile as tile
from concourse.bass_utils import run_bass_kernel_spmd

B, S, D, H = 4, 4096, 1024, 128
NCORES = 8
SK = S // 2  # keys per core (2048)
RB = 512  # rows per projection block
NRB = S // RB  # 8 query blocks
NKRB = SK // RB  # 4 key blocks
QC = 1024  # queries per attention chunk
NQC = S // QC  # 4 chunks
NKB = SK // 128  # 16 key blocks of 128
NDC = D // 128  # 8 contraction chunks

F32 = mybir.dt.float32
F16 = mybir.dt.float16

_CACHE = {}


def build_nc():
    nc = bacc.Bacc("TRN2", target_bir_lowering=False, debug=False)

    xt_d = nc.dram_tensor("xt", [D, S], F16, kind="ExternalInput")
    # weights host-preswizzled to [128, NDC*H]: row p, chunk c = W[c*128+p, :]
    wq_d = nc.dram_tensor("wq", [128, NDC * H], F16, kind="ExternalInput")
    wk_d = nc.dram_tensor("wk", [128, NDC * H], F16, kind="ExternalInput")
    wv_d = nc.dram_tensor("wv", [128, NDC * H], F16, kind="ExternalInput")
    ident_d = nc.dram_tensor("ident", [128, 128], F16, kind="ExternalInput")
    ones_d = nc.dram_tensor("ones", [128, 1], F16, kind="ExternalInput")
    # partial (key-shard) unnormalized out^T [h, q] and denominators l [1, q]
    outT_d = nc.dram_tensor("outT", [H, S], F32, kind="ExternalOutput")
    l_d = nc.dram_tensor("l", [1, S], F32, kind="ExternalOutput")

    scale = 1.0 / math.sqrt(H)

    with tile.TileContext(nc) as tc:
        with (
            tc.tile_pool(name="const", bufs=1) as constp,
            tc.tile_pool(name="persist", bufs=1) as persist,
            tc.tile_pool(name="attn", bufs=6) as attn_pool,
            tc.tile_pool(name="fin", bufs=2) as fin_pool,
            tc.tile_pool(name="ps_p", bufs=1, space="PSUM") as ps_p,
            tc.tile_pool(name="ps_s", bufs=2, space="PSUM") as ps_s,
            tc.tile_pool(name="ps_o", bufs=1, space="PSUM") as ps_o,
        ):
            # ---- DMA, ordered for the critical path ----
            w_sb = {}
            for name in ("wq", "wk", "wv"):
                w_sb[name] = constp.tile([128, NDC, H], F16, name=f"{name}_sb")

            def load_w(name):
                nc.sync.dma_start(
                    w_sb[name][:],
                    {"wq": wq_d, "wk": wk_d, "wv": wv_d}[name]
                    .ap()
                    .rearrange("p (c h) -> p c h", c=NDC),
                )

            xt_sb = persist.tile([128, NDC, S], F16, name="xt_sb")

            def load_slice(g):
                nc.sync.dma_start(
                    xt_sb[:, :, g * RB : (g + 1) * RB],
                    xt_d.ap()[:, g * RB : (g + 1) * RB].rearrange(
                        "(c p) s -> p c s", p=128
                    ),
                )

            ident = constp.tile([128, 128], F16, name="ident_sb")
            ones = constp.tile([128, 1], F16, name="ones_sb")

            load_w("wq")
            load_slice(0)
            load_w("wk")
            load_w("wv")
            nc.sync.dma_start(ident[:], ident_d.ap())
            nc.sync.dma_start(ones[:], ones_d.ap())
            for g in range(1, NRB):
                load_slice(g)

            # ---- persistent activations ----
            qt_sb = persist.tile([128, S], F16, name="qt_sb")  # [h, q] all q
            kt_sb = persist.tile([128, SK], F16, name="kt_sb")  # [h, k] own
            v_sb = persist.tile([128, NKB, H], F16, name="v_sb")  # own keys
            vt_sb = persist.tile([128, SK], F16, name="vt_sb")  # staging

            # preload the exp table during the input DMA
            warm = constp.tile([1, 1], F32, name="warm_sb")
            nc.scalar.activation(
                warm[:], ones[0:1, :], mybir.ActivationFunctionType.Exp
            )
            # HAM warm-up: ~3.5us of dummy matmuls on the just-arrived wq so
            # the PE clock is at 2.4 GHz when the real matmuls start
            warm_ps = ps_p.tile([128, 128], F32, tag="proj")
            for i in range(26):
                nc.tensor.matmul(
                    warm_ps[:],
                    w_sb["wq"][:, 0, :],
                    w_sb["wq"][:, 0, :],
                    start=(i == 0),
                    stop=(i == 25),
                )

            def project(wname, dst_sb, rb, pool, tag, width):
                """One 512-row projection block through one PSUM bank."""
                ps = pool.tile([128, width], F32, tag=tag)
                for dc in range(NDC):
                    nc.tensor.matmul(
                        ps[:, 0:RB],
                        w_sb[wname][:, dc, :],
                        xt_sb[:, dc, rb * RB : (rb + 1) * RB],
                        start=(dc == 0),
                        stop=(dc == NDC - 1),
                    )
                nc.vector.tensor_copy(dst_sb[:, rb * RB : (rb + 1) * RB], ps[:, 0:RB])

            def v_transpose(g):
                v_ps = ps_p.tile([128, RB], F16, tag="proj")
                for s in range(4):
                    nc.tensor.transpose(
                        v_ps[:, s * 128 : (s + 1) * 128],
                        vt_sb[:, g * RB + s * 128 : g * RB + (s + 1) * 128],
                        ident[:],
                    )
                nc.vector.tensor_copy(
                    v_sb[:, g * 4 : (g + 1) * 4, :].rearrange("p a b -> p (a b)"),
                    v_ps[:, 0 : 4 * H],
                )

            # Front: blocks attention chunk 0 needs, accumulated in parallel
            # on idle attention PSUM slots.
            project("wq", qt_sb, 0, ps_s, "st", QC)
            project("wk", kt_sb, 0, ps_s, "st", QC)
            project("wv", vt_sb, 0, ps_o, "outT", QC)
            project("wq", qt_sb, 1, ps_o, "l", 512)
            v_transpose(0)
            # Rest through the 1-bank proj slot; the scheduler overlaps with
            # the attention chain.
            for g in range(1, NKRB):
                project("wk", kt_sb, g, ps_p, "proj", RB)
                project("wv", vt_sb, g, ps_p, "proj", RB)
                v_transpose(g)
                project("wq", qt_sb, g + 1, ps_p, "proj", RB)
            for rb in range(NKRB + 1, NRB):
                project("wq", qt_sb, rb, ps_p, "proj", RB)

            # ---- attention (software-pipelined by one kb) ----
            for qcidx in range(NQC):
                outT_ps = ps_o.tile([128, QC], F32, tag="outT")
                # both 512-halves of l packed into ONE psum bank (partition 0
                # and partition 32 via tile_position col 32)
                l_ps = ps_o.tile([64, 512], F32, tag="l")
                at_tiles = {}

                def score(kb):
                    st_ps = ps_s.tile([128, QC], F32, tag="st")
                    for h in range(QC // 512):
                        nc.tensor.matmul(
                            st_ps[:, h * 512 : (h + 1) * 512],
                            kt_sb[:, kb * 128 : (kb + 1) * 128],
                            qt_sb[
                                :, qcidx * QC + h * 512 : qcidx * QC + (h + 1) * 512
                            ],
                            start=True,
                            stop=True,
                        )
                    at_sb = attn_pool.tile([128, QC], F16, tag="at")
                    nc.scalar.activation(
                        at_sb[:],
                        st_ps[:],
                        mybir.ActivationFunctionType.Exp,
                        scale=scale,
                    )
                    at_tiles[kb] = at_sb

                def accum(kb):
                    at_sb = at_tiles.pop(kb)
                    for h in range(QC // 512):
                        nc.tensor.matmul(
                            outT_ps[:, h * 512 : (h + 1) * 512],
                            v_sb[:, kb, :],
                            at_sb[:, h * 512 : (h + 1) * 512],
                            start=(kb == 0),
                            stop=(kb == NKB - 1),
                        )
                    for h in range(QC // 512):
                        nc.tensor.matmul(
                            l_ps[h * 32 : h * 32 + 1, :],
                            ones[:],
                            at_sb[:, h * 512 : (h + 1) * 512],
                            start=(kb == 0),
                            stop=(kb == NKB - 1),
                            tile_position=(0, h * 32),
                        )

                score(0)
                for kb in range(1, NKB):
                    score(kb)
                    accum(kb - 1)
                accum(NKB - 1)

                # evacuate partial outT and row-sums to HBM
                outT_sb = fin_pool.tile([128, QC], F32, tag="outT_sb")
                nc.vector.tensor_copy(outT_sb[:], outT_ps[:])
                nc.sync.dma_start(
                    outT_d.ap()[:, qcidx * QC : (qcidx + 1) * QC], outT_sb[:]
                )
                l_sb = fin_pool.tile([1, QC], F32, tag="l_sb")
                nc.vector.tensor_copy(l_sb[:, 0:512], l_ps[0:1, :])
                nc.vector.tensor_copy(l_sb[:, 512:1024], l_ps[32:33, :])
                nc.sync.dma_start(l_d.ap()[:, qcidx * QC : (qcidx + 1) * QC], l_sb[:])

    nc.compile()
    return nc


def _get_nc():
    if "nc" not in _CACHE:
        _CACHE["nc"] = build_nc()
    return _CACHE["nc"]


def _swizzle_w(W):
    # [D, H] -> [128, NDC*H]: row p, chunk c holds W[c*128+p, :]
    W = np.asarray(W, dtype=np.float16)
    return np.ascontiguousarray(
        W.reshape(NDC, 128, H).transpose(1, 0, 2).reshape(128, NDC * H)
    )


def make_in_maps(inputs, Wq, Wk, Wv):
    inputs = np.asarray(inputs, dtype=np.float32)
    Wq = _swizzle_w(Wq)
    Wk = _swizzle_w(Wk)
    Wv = _swizzle_w(Wv)
    ident = np.eye(128, dtype=np.float16)
    ones = np.ones((128, 1), dtype=np.float16)

    in_maps = []
    for c in range(NCORES):
        b, kh = divmod(c, 2)
        xb = inputs[b]
        # own key-half rows first; queries follow the same permutation
        xk = np.concatenate(
            [xb[kh * SK : (kh + 1) * SK], xb[(1 - kh) * SK : (2 - kh) * SK]], axis=0
        )
        xt = np.ascontiguousarray(xk.T.astype(np.float16))  # [D, S] fp16
        in_maps.append(
            {
                "xt": xt,
                "wq": Wq,
                "wk": Wk,
                "wv": Wv,
                "ident": ident,
                "ones": ones,
            }
        )
    return in_maps


def kernel(inputs, Wq, Wk, Wv):
    nc = _get_nc()
    in_maps = make_in_maps(inputs, Wq, Wk, Wv)

    res = run_bass_kernel_spmd(nc, in_maps, core_ids=list(range(NCORES)))

    out = np.empty((B, S, H), dtype=np.float32)
    for b in range(B):
        num = np.zeros((H, S), dtype=np.float32)
        den = np.zeros((1, S), dtype=np.float32)
        for kh in range(2):
            c = 2 * b + kh
            outT = res.results[c]["outT"]  # [H, S], query order permuted
            l = res.results[c]["l"]  # [1, S]
            # queries were ordered [kh-half, other-half]; map back
            perm = np.concatenate(
                [
                    np.arange(kh * SK, (kh + 1) * SK),
                    np.arange((1 - kh) * SK, (2 - kh) * SK),
                ]
            )
            num[:, perm] += outT
            den[:, perm] += l
        out[b] = (num / den).T
    return out


# revision 35
# speedup vs baseline: 1.9098x; 1.0714x over previous
"""Single-head attention kernel for Trainium2 (Bass/Tile), 8 NeuronCores.

Problem: B=4, S=4096, D=1024, H=128 fp32.
    q,k,v = x @ W{q,k,v};  out = softmax(q k^T / sqrt(H)) @ v

Sharding: 8 cores = (batch b, KEY-half kh).  Each core computes PARTIAL
attention for all 4096 queries over its 2048 keys; the host combines the
two partial results per batch: out = (outT_0 + outT_1) / (l_0 + l_1)
(unnormalized value-sums and softmax denominators add across key shards).
The host permutes each core's x rows so its key rows come first and
transposes/casts to xT [D, S] fp16.  Query order follows the same
permutation; the host maps it back when combining.

fp16 (e5m10) everywhere on the matmul operands: 2-byte operands stream
through the PE at 1 column/cycle @ 2.4 GHz (4-byte fp32/fp32r streams at
half rate) and its 10-bit mantissa keeps end-to-end error ~5e-4 (all
tensors here are O(1)).  All accumulation is fp32 in PSUM.

Per-core dataflow (PSUM: 1 bank proj + 4 banks scores + 2 banks outT +
1 bank row-sums = 8):
  1. xT k-slices DMA'd in (slice-major: each arriving 512-row slice
     completes projection blocks immediately); DMA order puts wq and
     slice 0 first; dummy matmuls warm the PE clock (HAM) meanwhile.
  2. Projections per 512-row block through rotating PSUM banks: the first
     blocks borrow idle attention PSUM slots to run in parallel.
     qT for all 8 query blocks; kT/vT only for the 4 own-key blocks;
     vT PE-transposed to v-natural.
  3. Scores TRANSPOSED, 1024-query chunks: sT[k,q] = kT(kb)^T @ qT -> PSUM.
     ScalarE exp reads sT from PSUM and writes attnT (fp16) straight to
     SBUF -- softmax PSUM evacuation fused into the exp.  No max
     subtraction needed: scores are ~N(0,1), fp32 exp is safe.
  4. outT[h,q] += v[kb]^T @ attnT;  l[q] += ones^T @ attnT  (fp32 PSUM,
     software-pipelined by one kb against the exp).
  5. Partial outT and l DMA'd out; host combines shards + normalizes.
"""

import math

import numpy as np

import concourse.bacc as bacc
import concourse.mybir as mybir
import concourse.tile as tile
from concourse.bass_utils import run_bass_kernel_spmd

B, S, D, H = 4, 4096, 1024, 128
NCORES = 8
SK = S // 2  # keys per core (2048)
RB = 512  # rows per projection block
NRB = S // RB  # 8 query blocks
NKRB = SK // RB  # 4 key blocks
QC = 1024  # queries per attention chunk
NQC = S // QC  # 4 chunks
NKB = SK // 128  # 16 key blocks of 128
NDC = D // 128  # 8 contraction chunks

F32 = mybir.dt.float32
F16 = mybir.dt.float16

_CACHE = {}


def build_nc():
    nc = bacc.Bacc("TRN2", target_bir_lowering=False, debug=False)

    xt_d = nc.dram_tensor("xt", [D, S], F16, kind="ExternalInput")
    # weights host-preswizzled to [128, NDC*H]: row p, chunk c = W[c*128+p, :]
    wq_d = nc.dram_tensor("wq", [128, NDC * H], F16, kind="ExternalInput")
    wk_d = nc.dram_tensor("wk", [128, NDC * H], F16, kind="ExternalInput")
    wv_d = nc.dram_tensor("wv", [128, NDC * H], F16, kind="ExternalInput")
    ident_d = nc.dram_tensor("ident", [128, 128], F16, kind="ExternalInput")
    ones_d = nc.dram_tensor("ones", [128, 1], F16, kind="ExternalInput")
    # partial (key-shard) unnormalized out^T [h, q] and denominators l [1, q]
    outT_d = nc.dram_tensor("outT", [H, S], F32, kind="ExternalOutput")
    l_d = nc.dram_tensor("l", [1, S], F32, kind="ExternalOutput")

    scale = 1.0 / math.sqrt(H)

    with tile.TileContext(nc) as tc:
        with (
            tc.tile_pool(name="const", bufs=1) as constp,
            tc.tile_pool(name="persist", bufs=1) as persist,
            tc.tile_pool(name="attn", bufs=6) as attn_pool,
            tc.tile_pool(name="fin", bufs=2) as fin_pool,
            tc.tile_pool(name="ps_p", bufs=1, space="PSUM") as ps_p,
            tc.tile_pool(name="ps_s", bufs=2, space="PSUM") as ps_s,
            tc.tile_pool(name="ps_o", bufs=1, space="PSUM") as ps_o,
        ):
            # ---- DMA, ordered for the critical path ----
            w_sb = {}
            for name in ("wq", "wk", "wv"):
                w_sb[name] = constp.tile([128, NDC, H], F16, name=f"{name}_sb")

            def load_w(name):
                nc.sync.dma_start(
                    w_sb[name][:],
                    {"wq": wq_d, "wk": wk_d, "wv": wv_d}[name]
                    .ap()
                    .rearrange("p (c h) -> p c h", c=NDC),
                )

            xt_sb = persist.tile([128, NDC, S], F16, name="xt_sb")

            def load_slice(g):
                nc.sync.dma_start(
                    xt_sb[:, :, g * RB : (g + 1) * RB],
                    xt_d.ap()[:, g * RB : (g + 1) * RB].rearrange(
                        "(c p) s -> p c s", p=128
                    ),
                )

            ident = constp.tile([128, 128], F16, name="ident_sb")
            ones = constp.tile([128, 1], F16, name="ones_sb")

            load_w("wq")
            load_slice(0)
            load_w("wk")
            load_w("wv")
            nc.sync.dma_start(ident[:], ident_d.ap())
            nc.sync.dma_start(ones[:], ones_d.ap())
            for g in range(1, NRB):
                load_slice(g)

            # ---- persistent activations ----
            qt_sb = persist.tile([128, S], F16, name="qt_sb")  # [h, q] all q
            kt_sb = persist.tile([128, SK], F16, name="kt_sb")  # [h, k] own
            v_sb = persist.tile([128, NKB, H], F16, name="v_sb")  # own keys
            vt_sb = persist.tile([128, SK], F16, name="vt_sb")  # staging

            # preload the exp table during the input DMA
            warm = constp.tile([1, 1], F32, name="warm_sb")
            nc.scalar.activation(
                warm[:], ones[0:1, :], mybir.ActivationFunctionType.Exp
            )
            # HAM warm-up: ~3.5us of dummy matmuls on the just-arrived wq so
            # the PE clock is at 2.4 GHz when the real matmuls start
            warm_ps = ps_p.tile([128, 128], F32, tag="proj")
            for i in range(26):
                nc.tensor.matmul(
                    warm_ps[:],
                    w_sb["wq"][:, 0, :],
                    w_sb["wq"][:, 0, :],
                    start=(i == 0),
                    stop=(i == 25),
                )

            def project(wname, dst_sb, rb, pool, tag, width):
                """One 512-row projection block through one PSUM bank."""
                ps = pool.tile([128, width], F32, tag=tag)
                for dc in range(NDC):
                    nc.tensor.matmul(
                        ps[:, 0:RB],
                        w_sb[wname][:, dc, :],
                        xt_sb[:, dc, rb * RB : (rb + 1) * RB],
                        start=(dc == 0),
                        stop=(dc == NDC - 1),
                    )
                nc.vector.tensor_copy(dst_sb[:, rb * RB : (rb + 1) * RB], ps[:, 0:RB])

            def v_transpose(g):
                v_ps = ps_p.tile([128, RB], F16, tag="proj")
                for s in range(4):
                    nc.tensor.transpose(
                        v_ps[:, s * 128 : (s + 1) * 128],
                        vt_sb[:, g * RB + s * 128 : g * RB + (s + 1) * 128],
                        ident[:],
                    )
                nc.vector.tensor_copy(
                    v_sb[:, g * 4 : (g + 1) * 4, :].rearrange("p a b -> p (a b)"),
                    v_ps[:, 0 : 4 * H],
                )

            # Front: blocks attention chunk 0 needs, accumulated in parallel
            # on idle attention PSUM slots.
            project("wq", qt_sb, 0, ps_s, "st", QC)
            project("wk", kt_sb, 0, ps_s, "st", QC)
            project("wv", vt_sb, 0, ps_o, "outT", QC)
            project("wq", qt_sb, 1, ps_o, "l", 512)
            v_transpose(0)
            # Rest through the 1-bank proj slot; the scheduler overlaps with
            # the attention chain.
            for g in range(1, NKRB):
                project("wk", kt_sb, g, ps_p, "proj", RB)
                project("wv", vt_sb, g, ps_p, "proj", RB)
                v_transpose(g)
                project("wq", qt_sb, g + 1, ps_p, "proj", RB)
            for rb in range(NKRB + 1, NRB):
                project("wq", qt_sb, rb, ps_p, "proj", RB)

            # ---- attention (software-pipelined by one kb) ----
            # The row-sums l use pair-summed attnT tiles: the (idle) DVE adds
            # at[2p]+at[2p+1] in fp16, halving the number of ones-matmuls the
            # PE has to stream.
            for qcidx in range(NQC):
                outT_ps = ps_o.tile([128, QC], F32, tag="outT")
                # both 512-halves of l packed into ONE psum bank (partition 0
                # and partition 32 via tile_position col 32)
                l_ps = ps_o.tile([64, 512], F32, tag="l")
                at_tiles = {}

                def score(kb):
                    st_ps = ps_s.tile([128, QC], F32, tag="st")
                    for h in range(QC // 512):
                        nc.tensor.matmul(
                            st_ps[:, h * 512 : (h + 1) * 512],
                            kt_sb[:, kb * 128 : (kb + 1) * 128],
                            qt_sb[
                                :, qcidx * QC + h * 512 : qcidx * QC + (h + 1) * 512
                            ],
                            start=True,
                            stop=True,
                        )
                    at_sb = attn_pool.tile([128, QC], F16, tag="at")
                    nc.scalar.activation(
                        at_sb[:],
                        st_ps[:],
                        mybir.ActivationFunctionType.Exp,
                        scale=scale,
                    )
                    at_tiles[kb] = at_sb

                def accum_av(kb):
                    at_sb = at_tiles[kb]
                    for h in range(QC // 512):
                        nc.tensor.matmul(
                            outT_ps[:, h * 512 : (h + 1) * 512],
                            v_sb[:, kb, :],
                            at_sb[:, h * 512 : (h + 1) * 512],
                            start=(kb == 0),
                            stop=(kb == NKB - 1),
                        )

                def accum_l(p):
                    a = at_tiles.pop(2 * p)
                    b = at_tiles.pop(2 * p + 1)
                    pair = attn_pool.tile([128, QC], F16, tag="pair", bufs=3)
                    nc.vector.tensor_add(pair[:], a[:], b[:])
                    for h in range(QC // 512):
                        nc.tensor.matmul(
                            l_ps[h * 32 : h * 32 + 1, :],
                            ones[:],
                            pair[:, h * 512 : (h + 1) * 512],
                            start=(p == 0),
                            stop=(p == NKB // 2 - 1),
                            tile_position=(0, h * 32),
                        )

                score(0)
                for kb in range(1, NKB):
                    score(kb)
                    accum_av(kb - 1)
                    if kb >= 2 and kb % 2 == 0:
                        accum_l((kb - 2) // 2)
                accum_av(NKB - 1)
                accum_l(NKB // 2 - 1)

                # evacuate partial outT and row-sums to HBM (l copies on the
                # otherwise-idle ScalarE)
                outT_sb = fin_pool.tile([128, QC], F32, tag="outT_sb")
                nc.vector.tensor_copy(outT_sb[:], outT_ps[:])
                nc.sync.dma_start(
                    outT_d.ap()[:, qcidx * QC : (qcidx + 1) * QC], outT_sb[:]
                )
                l_sb = fin_pool.tile([1, QC], F32, tag="l_sb")
                nc.scalar.copy(l_sb[:, 0:512], l_ps[0:1, :])
                nc.scalar.copy(l_sb[:, 512:1024], l_ps[32:33, :])
                nc.sync.dma_start(l_d.ap()[:, qcidx * QC : (qcidx + 1) * QC], l_sb[:])

    nc.compile()
    return nc


def _get_nc():
    if "nc" not in _CACHE:
        _CACHE["nc"] = build_nc()
    return _CACHE["nc"]


def _swizzle_w(W):
    # [D, H] -> [128, NDC*H]: row p, chunk c holds W[c*128+p, :]
    W = np.asarray(W, dtype=np.float16)
    return np.ascontiguousarray(
        W.reshape(NDC, 128, H).transpose(1, 0, 2).reshape(128, NDC * H)
    )


def make_in_maps(inputs, Wq, Wk, Wv):
    inputs = np.asarray(inputs, dtype=np.float32)
    Wq = _swizzle_w(Wq)
    Wk = _swizzle_w(Wk)
    Wv = _swizzle_w(Wv)
    ident = np.eye(128, dtype=np.float16)
    ones = np.ones((128, 1), dtype=np.float16)

    in_maps = []
    for c in range(NCORES):
        b, kh = divmod(c, 2)
        xb = inputs[b]
        # own key-half rows first; queries follow the same permutation
        xk = np.concatenate(
            [xb[kh * SK : (kh + 1) * SK], xb[(1 - kh) * SK : (2 - kh) * SK]], axis=0
        )
        xt = np.ascontiguousarray(xk.T.astype(np.float16))  # [D, S] fp16
        in_maps.append(
            {
                "xt": xt,
                "wq": Wq,
                "wk": Wk,
                "wv": Wv,
                "ident": ident,
                "ones": ones,
            }
        )
    return in_maps


def kernel(inputs, Wq, Wk, Wv):
    nc = _get_nc()
    in_maps = make_in_maps(inputs, Wq, Wk, Wv)

    res = run_bass_kernel_spmd(nc, in_maps, core_ids=list(range(NCORES)))

    out = np.empty((B, S, H), dtype=np.float32)
    for b in range(B):
        num = np.zeros((H, S), dtype=np.float32)
        den = np.zeros((1, S), dtype=np.float32)
        for kh in range(2):
            c = 2 * b + kh
            outT = res.results[c]["outT"]  # [H, S], query order permuted
            l = res.results[c]["l"]  # [1, S]
            # queries were ordered [kh-half, other-half]; map back
            perm = np.concatenate(
                [
                    np.arange(kh * SK, (kh + 1) * SK),
                    np.arange((1 - kh) * SK, (2 - kh) * SK),
                ]
            )
            num[:, perm] += outT
            den[:, perm] += l
        out[b] = (num / den).T
    return out
